# revision 26
# baseline (speedup 1.0000x reference)
"""GCN layer kernel for Trainium2, SPMD over 8 NeuronCores.

Reference computation (all fp32):
    adj_hat = rownorm(adj + I)                      # [N, N]
    out     = adj_hat @ (X @ W) + bias              # X: [N, T, A]

Sharding: T (time) axis split across 8 cores; adj/W/bias replicated.

bf16 I/O: the correctness gate is rel_err < 2e-2 and the full-bf16
datapath measures 4e-3, so X and out travel as bf16 - HBM traffic per
core drops 67MB -> 33.5MB (DMA was 91% busy at fp32). bf16 also makes
every matmul 1 cyc/col at any width and enables FWL weight loads that
hide LDWEIGHTS under the previous matmul.

Node indices are PARITY-chunked (chunk c holds nodes {2i+c}) rather
than half-chunked, so the adjacency loads as ONE contiguous 256KB DMA
([128 part, 2KB]: partition p <- rows 2p, 2p+1). The natural-layout
alternative ([128, 1KB-strided] x2) is descriptor-bound (~16GB/s) and
kept the whole setup chain - and therefore the first matmul - waiting
~20us. The bias broadcast tile is built on-chip with a rank-1 matmul
for the same reason.

Per-core kernel (T_SH = 256 time steps, time blocks of tb=16):
  setup: the G1 critical path is only adj DMA -> 4 PE parity-block
    transposes -> (+I, bf16 cast) -> adjT [n, m] (m-cols ordered (j, p)
    to match the output layout). The serial rowsum/reciprocal chain and
    the bias broadcast run AFTER, off the path to the first matmul; the
    1/deg row normalization is applied in the epilogue as a
    per-partition scalar (v1-style).
  per pair of time steps (2 t per PSUM bank, amortizes copy fixed cost):
    G1: ypt2[a, (t2 m)] = matmul(lhsT=X_t[n,a] bf16, rhs=adjT[n,m])
        x2 parity chunks x2 t -> one [128,512] PSUM bank
    ys2 = bf16(ypt2)                          (one ACT copy per 2 t)
    G2: ops2[m, (c t2 o)] = matmul(lhsT=ys2[a, m-chunk], rhs=W[a,o])
        x2 chunks x2 t -> one [128,512] PSUM bank
    out = bf16(r[m]*ops2 + bias)              (two DVE stt per 2 t)
  Each HWDGE ring tops out ~200-300GB/s, so X loads alternate between
  the sync and scalar rings and stores ride gpsimd; the last blocks'
  stores fan out across rings (all loads are emitted by then, so no
  head-of-line blocking) and the final store is split in half across
  two rings to shorten the drain tail.
Host: converts X/W to bf16, slices T, and upcasts the bf16 output back
to fp32.
"""

import os
import sys

import numpy as np

for _p in ("/opt/trn_rl_repo", "/root/.axon_site/_ro/trn_rl_repo"):
    if os.path.isdir(_p) and _p not in sys.path:
        sys.path.insert(0, _p)

import concourse.bass as bass
import concourse.mybir as mybir
import concourse.tile as tile
from concourse import bacc
from concourse.bass_utils import run_bass_kernel_spmd
from concourse.masks import make_identity

N_NODES = 256
N_TIMES = 2048
N_FEAT = 128
N_CORES = 8
T_SH = N_TIMES // N_CORES  # 256 time steps per core
P = 128  # partitions
NCH = N_NODES // P  # 2 node parity chunks

F32 = mybir.dt.float32
BF16 = mybir.dt.bfloat16


def _gcn_body(tc, out, x, adj, w, b, t_sh, tb):
    nc = tc.nc
    nblk = t_sh // tb
    ngrp = tb // 2  # 2 time steps per PSUM bank

    from contextlib import ExitStack

    with ExitStack() as ctx:
        const = ctx.enter_context(tc.tile_pool(name="const", bufs=1))
        setup = ctx.enter_context(tc.tile_pool(name="setup", bufs=1))

        # --- setup DMAs first, at the HEAD of the two hardware rings
        # (sync/scalar), ahead of the X prefetch. The gpsimd "ring" is a
        # software-dynamic queue whose Q7-generated descriptors only start
        # flowing ~12us in - too late for the setup chain. adj is ONE
        # contiguous 256KB transfer: partition p <- rows 2p, 2p+1.
        a_sb = setup.tile([P, 2, N_NODES], F32, name="a", tag="a")
        nc.sync.dma_start(out=a_sb, in_=adj.rearrange("(p j) n -> p j n", j=2))

        w_sb = const.tile([P, N_FEAT], BF16)
        nc.scalar.dma_start(out=w_sb, in_=w)

        # bias lands as a single [1, 128] row; the partition broadcast is
        # done on-chip (rank-1 matmul) - a [0,128]-broadcast DMA would be
        # descriptor-bound and hog the queue for ~10us.
        bias_row = setup.tile([1, N_FEAT], F32, name="brow", tag="brow")
        bias_row_ap = bass.AP(
            tensor=b.tensor, offset=b.offset, ap=[[0, 1], b.ap[0]]
        )
        nc.scalar.dma_start(out=bias_row, in_=bias_row_ap)

        ident = const.tile([P, P], F32)
        make_identity(nc, ident)

        # bias replicated across partitions and duplicated over t2 so one
        # DVE scalar_tensor_tensor per (chunk, pair) covers its PSUM slice
        bias_bc3 = const.tile([P, 2, N_FEAT], F32)

        # adjT_hat[n, m] = (adj[m, n] + I) / deg[m]: n on partitions in
        # parity order (partition q of chunk k <-> node 2q+k), m in the
        # free dim ordered (j, p) <-> node 2p+j.
        adjT = [
            const.tile([P, N_NODES], BF16, name=f"adjT{c}", tag=f"adjT{c}")
            for c in range(NCH)
        ]

        # Main-loop SBUF pools are created BEFORE the setup scratch pool's
        # remaining tiles so their addresses don't alias the setup chain.
        xp = ctx.enter_context(tc.tile_pool(name="xp", bufs=6))
        op = ctx.enter_context(tc.tile_pool(name="op", bufs=3))
        ysb = ctx.enter_context(tc.tile_pool(name="ysb", bufs=ngrp + 2))

        # [n, t, a] viewed as [n//2, n%2, t, a]: parity chunking, one 1MB
        # DMA moves both parity chunks of a time block (4KB runs)
        x4 = x.rearrange("(n c) t a -> n c t a", c=NCH)
        out4 = out.rearrange("(m c) t a -> m c t a", c=NCH)

        # Each HWDGE ring tops out around ~200GB/s. Directions stay
        # DISJOINT per ring (a store descriptor waiting on its epilogue
        # would head-of-line block later loads in the same FIFO): loads
        # alternate sync/scalar, stores go to gpsimd except the tail.
        load_eng = [nc.sync, nc.scalar]

        def load_x(blk):
            t0 = blk * tb
            xtc = xp.tile([P, NCH, tb, N_FEAT], BF16, name=f"x_{blk}", tag="x")
            if blk == 0 and tb >= 2:
                # block 0 gates the first matmul: split it across both
                # rings so it lands ~2x sooner
                h = tb // 2
                nc.sync.dma_start(
                    out=xtc[:, :, 0:h, :], in_=x4[:, :, t0 : t0 + h, :]
                )
                nc.scalar.dma_start(
                    out=xtc[:, :, h:tb, :], in_=x4[:, :, t0 + h : t0 + tb, :]
                )
            else:
                load_eng[blk % 2].dma_start(
                    out=xtc, in_=x4[:, :, t0 : t0 + tb, :]
                )
            return xtc

        PF = 4  # prefetch depth (< xp bufs)
        prefetched = [load_x(blk) for blk in range(min(PF, nblk))]

        with tc.tile_pool(name="setup_ps", bufs=1, space="PSUM") as setup_ps:
            # The G1 critical path is ONLY: adj DMA -> 4 PE transposes ->
            # (+I on diag blocks) -> bf16 copies. The row normalization is
            # applied later, in the epilogue, as a per-partition scalar -
            # keeping the serial rowsum/reciprocal chain OFF the path to
            # the first matmul.
            # transpose parity blocks: B_jk[p, q] = adj[2p+j, 2q+k]
            # -> adjT[k][q, j*128+p]; diag blocks (j==k) get +I via the
            # DVE add that also does the psum->bf16 cast
            a_blk = a_sb.rearrange("p j (q k) -> p j q k", k=NCH)
            for k in range(NCH):
                for j in range(NCH):
                    tp = setup_ps.tile([P, P], F32, name="tp", tag="tp")
                    nc.tensor.transpose(tp, a_blk[:, j, :, k], ident)
                    dst = adjT[k][:, j * P : (j + 1) * P]
                    if j == k:
                        nc.vector.tensor_add(dst, tp, ident)
                    else:
                        nc.scalar.copy(dst, tp)

            # off-critical-path: r[p, j] = 1 / (1 + sum_n adj[2p+j, n])
            # (consumed by the first epilogue, ~10us later)
            dg2 = setup.tile([P, NCH], F32, name="dg2", tag="dg2")
            nc.vector.reduce_sum(dg2, a_sb, axis=mybir.AxisListType.X)
            nc.vector.tensor_scalar_add(dg2, dg2, 1.0)
            r2 = setup.tile([P, NCH], F32, name="r2", tag="r2")
            nc.vector.reciprocal(r2, dg2)

            # broadcast bias across partitions: psum[i, o] = bias[o]
            ones1 = setup.tile([1, P], F32, name="ones1", tag="ones1")
            nc.vector.memset(ones1, 1.0)
            bps = setup_ps.tile([P, N_FEAT], F32, name="bps", tag="bps")
            nc.tensor.matmul(bps, ones1, bias_row, start=True, stop=True)
            for rep in range(2):
                nc.vector.tensor_copy(bias_bc3[:, rep, :], bps)

        yps = ctx.enter_context(tc.tile_pool(name="yps", bufs=3, space="PSUM"))
        ops = ctx.enter_context(tc.tile_pool(name="ops", bufs=3, space="PSUM"))

        for blk in range(nblk):
            t0 = blk * tb
            # sliding-window prefetch: issue the load PF blocks ahead NOW,
            # before this block's store is emitted
            if blk + PF < nblk:
                prefetched.append(load_x(blk + PF))
            xt = prefetched[blk]
            ot = op.tile([P, NCH, tb, N_FEAT], BF16, name=f"o_{blk}", tag="o")
            # Phase 1: aggregation matmuls, 2 time steps per PSUM bank, one
            # ACT psum->sbuf bf16 copy per pair. Back-to-back GEMM1s keep
            # PE busy while the copies land.
            ys_list = []
            for gi in range(ngrp):
                ypt2 = yps.tile([P, 2, N_NODES], F32, name="ypt2", tag="y")
                for q in range(2):
                    ti = gi * 2 + q
                    for ck in range(NCH):
                        nc.tensor.matmul(
                            ypt2[:, q, :],
                            xt[:, ck, ti, :],
                            adjT[ck],
                            start=(ck == 0),
                            stop=(ck == NCH - 1),
                        )
                ys2 = ysb.tile([P, 2, N_NODES], BF16, name=f"ys{gi}", tag="ys")
                nc.scalar.copy(ys2, ypt2)
                ys_list.append(ys2)
            # Phase 2: feature-transform matmuls into a (c, t2, o) PSUM
            # bank; the epilogue applies the row normalization as a
            # per-partition scalar (out node m=2p+mc -> scalar r2[p, mc])
            # fused with the bias add and bf16 cast: one DVE stt per
            # (chunk, pair)
            for gi in range(ngrp):
                opt2 = ops.tile([P, NCH, 2, N_FEAT], F32, name="opt2", tag="op")
                for mc in range(NCH):
                    for q in range(2):
                        nc.tensor.matmul(
                            opt2[:, mc, q, :],
                            ys_list[gi][:, q, mc * P : (mc + 1) * P],
                            w_sb,
                            start=True,
                            stop=True,
                        )
                tt0 = gi * 2
                for mc in range(NCH):
                    nc.vector.scalar_tensor_tensor(
                        out=ot[:, mc, tt0 : tt0 + 2, :],
                        in0=opt2[:, mc, :, :],
                        scalar=r2[:, mc : mc + 1],
                        in1=bias_bc3,
                        op0=mybir.AluOpType.mult,
                        op1=mybir.AluOpType.add,
                    )
            # stores: gpsimd until the tail; the very last block is split
            # in half across sync+scalar so the drain overlaps
            if blk == nblk - 1 and tb >= 2:
                h = tb // 2
                nc.sync.dma_start(
                    out=out4[:, :, t0 : t0 + h, :], in_=ot[:, :, 0:h, :]
                )
                nc.scalar.dma_start(
                    out=out4[:, :, t0 + h : t0 + tb, :], in_=ot[:, :, h:tb, :]
                )
            elif blk >= nblk - 4:
                eng = [nc.sync, nc.scalar, nc.gpsimd][blk % 3]
                eng.dma_start(out=out4[:, :, t0 : t0 + tb, :], in_=ot)
            else:
                nc.gpsimd.dma_start(out=out4[:, :, t0 : t0 + tb, :], in_=ot)


def build(t_sh=T_SH, tb=16):
    """Build + compile the per-core Bass module."""
    nc = bacc.Bacc(
        "TRN2", target_bir_lowering=False, debug=False, num_devices=N_CORES
    )
    x = nc.dram_tensor("node_feats", [N_NODES, t_sh, N_FEAT], BF16, kind="ExternalInput").ap()
    adj = nc.dram_tensor("adj_matrix", [N_NODES, N_NODES], F32, kind="ExternalInput").ap()
    w = nc.dram_tensor("weight", [N_FEAT, N_FEAT], BF16, kind="ExternalInput").ap()
    b = nc.dram_tensor("bias", [N_FEAT], F32, kind="ExternalInput").ap()
    out = nc.dram_tensor("out", [N_NODES, t_sh, N_FEAT], BF16, kind="ExternalOutput").ap()
    with tile.TileContext(nc) as tc:
        _gcn_body(tc, out, x, adj, w, b, t_sh, tb)
    nc.compile()
    return nc


_built_nc = None


def _get_nc():
    global _built_nc
    if _built_nc is None:
        _built_nc = build()
    return _built_nc


def _run(node_feats, adj_matrix, weight, bias, trace=False, tmpdir=None):
    import ml_dtypes

    nc = _get_nc()
    node_feats = np.ascontiguousarray(node_feats, dtype=np.float32)
    adj_matrix = np.ascontiguousarray(adj_matrix, dtype=np.float32)
    weight = np.ascontiguousarray(weight, dtype=np.float32).astype(
        ml_dtypes.bfloat16
    )
    bias = np.ascontiguousarray(bias, dtype=np.float32)
    in_maps = [
        {
            "node_feats": np.ascontiguousarray(
                node_feats[:, c * T_SH : (c + 1) * T_SH, :]
            ).astype(ml_dtypes.bfloat16),
            "adj_matrix": adj_matrix,
            "weight": weight,
            "bias": bias,
        }
        for c in range(N_CORES)
    ]
    res = run_bass_kernel_spmd(
        nc, in_maps, list(range(N_CORES)), trace=trace, tmpdir=tmpdir
    )
    out = np.concatenate(
        [res.results[c]["out"] for c in range(N_CORES)], axis=1
    ).astype(np.float32)
    return out, res


def kernel(node_feats, adj_matrix, weight, bias):
    out, _ = _run(node_feats, adj_matrix, weight, bias)
    return out


# revision 29
# speedup vs baseline: 1.3046x; 1.3046x over previous
"""GCN layer kernel for Trainium2, SPMD over 8 NeuronCores.

Reference computation (all fp32):
    adj_hat = rownorm(adj + I)                      # [N, N]
    out     = adj_hat @ (X @ W) + bias              # X: [N, T, A]

Sharding: T (time) axis split across 8 cores; adj/W/bias replicated.

bf16 I/O: the correctness gate is rel_err < 2e-2 and the full-bf16
datapath measures 4e-3, so X and out travel as bf16 - HBM traffic per
core drops 67MB -> 33.5MB (DMA was 91% busy at fp32). bf16 also makes
every matmul 1 cyc/col at any width and enables FWL weight loads that
hide LDWEIGHTS under the previous matmul.

Node indices are PARITY-chunked (chunk c holds nodes {2i+c}) rather
than half-chunked, so the adjacency loads as ONE contiguous 256KB DMA
([128 part, 2KB]: partition p <- rows 2p, 2p+1). The natural-layout
alternative ([128, 1KB-strided] x2) is descriptor-bound (~16GB/s) and
kept the whole setup chain - and therefore the first matmul - waiting
~20us. The bias broadcast tile is built on-chip with a rank-1 matmul
for the same reason.

Per-core kernel (T_SH = 256 time steps, time blocks of tb=16):
  setup: the G1 critical path is only adj DMA -> 4 PE parity-block
    transposes -> (+I, bf16 cast) -> adjT [n, m] (m-cols ordered (j, p)
    to match the output layout). The serial rowsum/reciprocal chain and
    the bias broadcast run AFTER, off the path to the first matmul; the
    1/deg row normalization is applied in the epilogue as a
    per-partition scalar (v1-style).
  per pair of time steps (2 t per PSUM bank, amortizes copy fixed cost):
    G1: ypt2[a, (t2 m)] = matmul(lhsT=X_t[n,a] bf16, rhs=adjT[n,m])
        x2 parity chunks x2 t -> one [128,512] PSUM bank
    ys2 = bf16(ypt2)                          (one ACT copy per 2 t)
    G2: ops2[m, (c t2 o)] = matmul(lhsT=ys2[a, m-chunk], rhs=W[a,o])
        x2 chunks x2 t -> one [128,512] PSUM bank
    out = bf16(r[m]*ops2 + bias)              (two DVE stt per 2 t)
  Each HWDGE ring tops out ~200-300GB/s, so X loads alternate between
  the sync and scalar rings and stores ride gpsimd; the last blocks'
  stores fan out across rings (all loads are emitted by then, so no
  head-of-line blocking) and the final store is split in half across
  two rings to shorten the drain tail.
Host: converts X/W to bf16, slices T, and upcasts the bf16 output back
to fp32.
"""

import os
import sys

import numpy as np

for _p in ("/opt/trn_rl_repo", "/root/.axon_site/_ro/trn_rl_repo"):
    if os.path.isdir(_p) and _p not in sys.path:
        sys.path.insert(0, _p)

import concourse.bass as bass
import concourse.mybir as mybir
import concourse.tile as tile
from concourse import bacc
from concourse.bass_utils import run_bass_kernel_spmd
from concourse.masks import make_identity

N_NODES = 256
N_TIMES = 2048
N_FEAT = 128
N_CORES = 8
T_SH = N_TIMES // N_CORES  # 256 time steps per core
P = 128  # partitions
NCH = N_NODES // P  # 2 node parity chunks

F32 = mybir.dt.float32
BF16 = mybir.dt.bfloat16


def _gcn_body(tc, out, x, adj, w, b, t_sh, tb):
    nc = tc.nc
    nblk = t_sh // tb
    ngrp = tb // 2  # 2 time steps per PSUM bank

    from contextlib import ExitStack

    with ExitStack() as ctx:
        const = ctx.enter_context(tc.tile_pool(name="const", bufs=1))
        setup = ctx.enter_context(tc.tile_pool(name="setup", bufs=1))

        # --- setup DMAs first, at the HEAD of the two hardware rings
        # (sync/scalar), ahead of the X prefetch. The gpsimd "ring" is a
        # software-dynamic queue whose Q7-generated descriptors only start
        # flowing ~12us in - too late for the setup chain. adj is ONE
        # contiguous 256KB transfer: partition p <- rows 2p, 2p+1.
        a_sb = setup.tile([P, 2, N_NODES], F32, name="a", tag="a")
        nc.sync.dma_start(out=a_sb, in_=adj.rearrange("(p j) n -> p j n", j=2))

        w_sb = const.tile([P, N_FEAT], BF16)
        nc.scalar.dma_start(out=w_sb, in_=w)

        # bias lands as a single [1, 128] row; the partition broadcast is
        # done on-chip (rank-1 matmul) - a [0,128]-broadcast DMA would be
        # descriptor-bound and hog the queue for ~10us.
        bias_row = setup.tile([1, N_FEAT], F32, name="brow", tag="brow")
        bias_row_ap = bass.AP(
            tensor=b.tensor, offset=b.offset, ap=[[0, 1], b.ap[0]]
        )
        nc.scalar.dma_start(out=bias_row, in_=bias_row_ap)

        ident = const.tile([P, P], F32)
        make_identity(nc, ident)

        # bias replicated across partitions and duplicated (c, t2) so ONE
        # DVE add per pair covers the whole [c, t2, o] PSUM bank (DVE ops
        # carry ~0.5us fixed cost each - op count is what matters)
        bias_bc3 = const.tile([P, NCH * 2, N_FEAT], F32)
        bias_bc = bias_bc3.rearrange("p (c q) o -> p c q o", c=NCH)

        # adjT_hat[n, m] = (adj[m, n] + I) / deg[m]: n on partitions in
        # parity order (partition q of chunk k <-> node 2q+k), m in the
        # free dim ordered (j, p) <-> node 2p+j.
        adjT = [
            const.tile([P, N_NODES], BF16, name=f"adjT{c}", tag=f"adjT{c}")
            for c in range(NCH)
        ]

        # Main-loop SBUF pools are created BEFORE the setup scratch pool's
        # remaining tiles so their addresses don't alias the setup chain.
        xp = ctx.enter_context(tc.tile_pool(name="xp", bufs=6))
        op = ctx.enter_context(tc.tile_pool(name="op", bufs=3))
        ysb = ctx.enter_context(tc.tile_pool(name="ysb", bufs=ngrp + 2))

        # [n, t, a] viewed as [n//2, n%2, t, a]: parity chunking, one 1MB
        # DMA moves both parity chunks of a time block (4KB runs)
        x4 = x.rearrange("(n c) t a -> n c t a", c=NCH)
        out4 = out.rearrange("(m c) t a -> m c t a", c=NCH)

        # Each HWDGE ring tops out around ~200GB/s. Directions stay
        # DISJOINT per ring (a store descriptor waiting on its epilogue
        # would head-of-line block later loads in the same FIFO): loads
        # alternate sync/scalar, stores go to gpsimd except the tail.
        load_eng = [nc.sync, nc.scalar]

        def load_x(blk):
            t0 = blk * tb
            xtc = xp.tile([P, NCH, tb, N_FEAT], BF16, name=f"x_{blk}", tag="x")
            if blk == 0 and tb >= 2:
                # block 0 gates the first matmul: split it across both
                # rings so it lands ~2x sooner
                h = tb // 2
                nc.sync.dma_start(
                    out=xtc[:, :, 0:h, :], in_=x4[:, :, t0 : t0 + h, :]
                )
                nc.scalar.dma_start(
                    out=xtc[:, :, h:tb, :], in_=x4[:, :, t0 + h : t0 + tb, :]
                )
            else:
                load_eng[blk % 2].dma_start(
                    out=xtc, in_=x4[:, :, t0 : t0 + tb, :]
                )
            return xtc

        PF = 4  # prefetch depth (< xp bufs)
        prefetched = [load_x(blk) for blk in range(min(PF, nblk))]

        with tc.tile_pool(name="setup_ps", bufs=1, space="PSUM") as setup_ps:
            # G1 critical path: adj DMA -> fused rowsum chain (3 DVE ops)
            # -> row scale -> 4 PE transposes -> bf16 casts. The +r*I on
            # the diagonal blocks rides the post-transpose cast (DVE add
            # instead of ACT copy), not a separate pass.
            # r[p, j] = 1 / (1 + sum_n adj[2p+j, n]), both j at once
            dg2 = setup.tile([P, NCH], F32, name="dg2", tag="dg2")
            nc.vector.reduce_sum(dg2, a_sb, axis=mybir.AxisListType.X)
            nc.vector.tensor_scalar_add(dg2, dg2, 1.0)
            r2 = setup.tile([P, NCH], F32, name="r2", tag="r2")
            nc.vector.reciprocal(r2, dg2)
            rdiag = []
            for j in range(NCH):
                nc.vector.tensor_scalar_mul(
                    a_sb[:, j, :], a_sb[:, j, :], r2[:, j : j + 1]
                )
                rd = setup.tile([P, P], F32, name=f"rd{j}", tag=f"rd{j}")
                nc.vector.tensor_scalar_mul(rd, ident, r2[:, j : j + 1])
                rdiag.append(rd)

            # transpose parity blocks: B_jk[p, q] = (adj*r)[2p+j, 2q+k]
            # -> adjT[k][q, j*128+p]; diag blocks get +diag(r) fused into
            # the psum->bf16 cast
            a_blk = a_sb.rearrange("p j (q k) -> p j q k", k=NCH)
            for k in range(NCH):
                for j in range(NCH):
                    tp = setup_ps.tile([P, P], F32, name="tp", tag="tp")
                    nc.tensor.transpose(tp, a_blk[:, j, :, k], ident)
                    dst = adjT[k][:, j * P : (j + 1) * P]
                    if j == k:
                        nc.vector.tensor_add(dst, tp, rdiag[j])
                    else:
                        nc.scalar.copy(dst, tp)

            # broadcast bias across partitions: psum[i, o] = bias[o]
            ones1 = setup.tile([1, P], F32, name="ones1", tag="ones1")
            nc.vector.memset(ones1, 1.0)
            bps = setup_ps.tile([P, N_FEAT], F32, name="bps", tag="bps")
            nc.tensor.matmul(bps, ones1, bias_row, start=True, stop=True)
            for rep in range(NCH * 2):
                nc.vector.tensor_copy(bias_bc3[:, rep, :], bps)

        yps = ctx.enter_context(tc.tile_pool(name="yps", bufs=3, space="PSUM"))
        ops = ctx.enter_context(tc.tile_pool(name="ops", bufs=3, space="PSUM"))

        for blk in range(nblk):
            t0 = blk * tb
            # sliding-window prefetch: issue the load PF blocks ahead NOW,
            # before this block's store is emitted
            if blk + PF < nblk:
                prefetched.append(load_x(blk + PF))
            xt = prefetched[blk]
            ot = op.tile([P, NCH, tb, N_FEAT], BF16, name=f"o_{blk}", tag="o")
            # Phase 1: aggregation matmuls, 2 time steps per PSUM bank, one
            # ACT psum->sbuf bf16 copy per pair. Back-to-back GEMM1s keep
            # PE busy while the copies land.
            ys_list = []
            for gi in range(ngrp):
                ypt2 = yps.tile([P, 2, N_NODES], F32, name="ypt2", tag="y")
                for q in range(2):
                    ti = gi * 2 + q
                    for ck in range(NCH):
                        nc.tensor.matmul(
                            ypt2[:, q, :],
                            xt[:, ck, ti, :],
                            adjT[ck],
                            start=(ck == 0),
                            stop=(ck == NCH - 1),
                        )
                ys2 = ysb.tile([P, 2, N_NODES], BF16, name=f"ys{gi}", tag="ys")
                nc.scalar.copy(ys2, ypt2)
                ys_list.append(ys2)
            # Phase 2: feature-transform matmuls into a (c, t2, o) PSUM
            # bank; the epilogue applies the row normalization as a
            # per-partition scalar (out node m=2p+mc -> scalar r2[p, mc])
            # fused with the bias add and bf16 cast: one DVE stt per
            # (chunk, pair)
            for gi in range(ngrp):
                opt2 = ops.tile([P, NCH, 2, N_FEAT], F32, name="opt2", tag="op")
                for mc in range(NCH):
                    for q in range(2):
                        nc.tensor.matmul(
                            opt2[:, mc, q, :],
                            ys_list[gi][:, q, mc * P : (mc + 1) * P],
                            w_sb,
                            start=True,
                            stop=True,
                        )
                tt0 = gi * 2
                nc.vector.tensor_add(
                    ot[:, :, tt0 : tt0 + 2, :], opt2, bias_bc
                )
            # stores: gpsimd until the tail; the very last block is split
            # in half across sync+scalar so the drain overlaps
            if blk == nblk - 1 and tb >= 2:
                h = tb // 2
                nc.sync.dma_start(
                    out=out4[:, :, t0 : t0 + h, :], in_=ot[:, :, 0:h, :]
                )
                nc.scalar.dma_start(
                    out=out4[:, :, t0 + h : t0 + tb, :], in_=ot[:, :, h:tb, :]
                )
            elif blk >= nblk - 4:
                eng = [nc.sync, nc.scalar, nc.gpsimd][blk % 3]
                eng.dma_start(out=out4[:, :, t0 : t0 + tb, :], in_=ot)
            else:
                nc.gpsimd.dma_start(out=out4[:, :, t0 : t0 + tb, :], in_=ot)


def build(t_sh=T_SH, tb=16):
    """Build + compile the per-core Bass module."""
    nc = bacc.Bacc(
        "TRN2", target_bir_lowering=False, debug=False, num_devices=N_CORES
    )
    x = nc.dram_tensor("node_feats", [N_NODES, t_sh, N_FEAT], BF16, kind="ExternalInput").ap()
    adj = nc.dram_tensor("adj_matrix", [N_NODES, N_NODES], F32, kind="ExternalInput").ap()
    w = nc.dram_tensor("weight", [N_FEAT, N_FEAT], BF16, kind="ExternalInput").ap()
    b = nc.dram_tensor("bias", [N_FEAT], F32, kind="ExternalInput").ap()
    out = nc.dram_tensor("out", [N_NODES, t_sh, N_FEAT], BF16, kind="ExternalOutput").ap()
    with tile.TileContext(nc) as tc:
        _gcn_body(tc, out, x, adj, w, b, t_sh, tb)
    nc.compile()
    return nc


_built_nc = None


def _get_nc():
    global _built_nc
    if _built_nc is None:
        _built_nc = build()
    return _built_nc


def _run(node_feats, adj_matrix, weight, bias, trace=False, tmpdir=None):
    import ml_dtypes

    nc = _get_nc()
    node_feats = np.ascontiguousarray(node_feats, dtype=np.float32)
    adj_matrix = np.ascontiguousarray(adj_matrix, dtype=np.float32)
    weight = np.ascontiguousarray(weight, dtype=np.float32).astype(
        ml_dtypes.bfloat16
    )
    bias = np.ascontiguousarray(bias, dtype=np.float32)
    in_maps = [
        {
            "node_feats": np.ascontiguousarray(
                node_feats[:, c * T_SH : (c + 1) * T_SH, :]
            ).astype(ml_dtypes.bfloat16),
            "adj_matrix": adj_matrix,
            "weight": weight,
            "bias": bias,
        }
        for c in range(N_CORES)
    ]
    res = run_bass_kernel_spmd(
        nc, in_maps, list(range(N_CORES)), trace=trace, tmpdir=tmpdir
    )
    out = np.concatenate(
        [res.results[c]["out"] for c in range(N_CORES)], axis=1
    ).astype(np.float32)
    return out, res


def kernel(node_feats, adj_matrix, weight, bias):
    out, _ = _run(node_feats, adj_matrix, weight, bias)
    return out
